# revision 32
# baseline (speedup 1.0000x reference)
"""Multi-head attention (RoPE) Trainium2 kernel, v4.

Problem: B=2, T=2048, D_MODEL=1024, 16 heads x d_k=64, fp32 in/out.

Sharding: tensor-parallel over heads. Core c owns heads 2c, 2c+1:
  - wq/wk/wv rows [128c, 128c+128)  (column-split of the projections)
  - wo columns [128c, 128c+128)     (row-split of the output projection)
Each core computes its two heads' attention, normalizes the softmax
ON DEVICE (reciprocal of the ones-column denominator folded into the
O^T eviction), then runs a single merged output projection over both
heads (contraction 128). The host only sums the 8 cores' f16 partial
outputs (the "all-reduce" of row-parallel wo).

Scheduling design (the scarce resource is PSUM: 8 banks):
  - AV accumulates in h2-halves: ps_o tiles are [65, 512] (1 bank x2),
    pass 1 (queries 0:512) runs inside the kt loop, pass 2 at the START
    of the next chunk (exp tiles are retained until then), so the Act
    engine is never starved while the PE runs the pass-2 block.
  - "mm" pool gets 3 slots [128,1024] (6 banks): 2 rotate the scores
    double-buffer, 1 carries backfill pieces (projection accumulations,
    V transposes, output-projection pieces) that keep the PE streaming
    continuously -- a dense stream keeps the PE at its 2.4 GHz pstate
    (it drops to 1.2 GHz ~ after any idle).
  - exp is the Act-engine pacer: 128 tiles x ~1.11us. Everything else
    hides underneath it.
"""

import sys

sys.path.insert(0, "/opt/trn_rl_repo")

import numpy as np

import concourse.bacc as bacc
import concourse.bass as bass
import concourse.tile as tile
from concourse import mybir
from concourse.masks import make_identity

F16 = mybir.dt.float16
F32 = mybir.dt.float32

B = 2
T = 2048
D = 1024
NTOK = B * T  # 4096
DK = 64
N_CORES = 8
QCH = 1024  # query chunk (per (b, qh))
KT_N = T // 128  # 16 key tiles per batch
Mul = mybir.AluOpType.mult
Add = mybir.AluOpType.add


def _build_body(tc, xT, wqT, wkT, wvT, woT, ropeA, ropeB, outT):
    nc = tc.nc
    Exp = mybir.ActivationFunctionType.Exp

    const = tc.alloc_tile_pool(name="const", bufs=1)
    psum = tc.alloc_tile_pool(name="psum", bufs=1, space="PSUM")

    # ---------------- persistent tiles / input DMA ----------------
    w_sb = {}
    wt = const.tile([128, 8, 128], F16, name="wqsb")
    nc.sync.dma_start(out=wt, in_=wqT.rearrange("(a p) m -> p a m", p=128))
    w_sb["wq"] = wt
    xs = [const.tile([128, 4096], F16, name=f"xs{k}") for k in range(8)]
    for k in range(8):  # t4=0 slices first so the first projection can start
        nc.sync.dma_start(out=xs[k][:, 0:1024], in_=xT[k * 128 : (k + 1) * 128, 0:1024])
    for nm, w in (("wk", wkT), ("wv", wvT)):
        wt = const.tile([128, 8, 128], F16, name=f"{nm}sb")
        nc.sync.dma_start(out=wt, in_=w.rearrange("(a p) m -> p a m", p=128))
        w_sb[nm] = wt
    wo_sb = const.tile([128, 1024], F16)
    nc.sync.dma_start(out=wo_sb, in_=woT)
    rA = const.tile([128, 2048], F16)  # rope tables, one batch wide
    nc.sync.dma_start(out=rA, in_=ropeA)
    rB = const.tile([128, 2048], F16)  # pre-swap-multiplied B table
    nc.sync.dma_start(out=rB, in_=ropeB)
    for t4 in range(1, 4):
        for k in range(8):
            cs = slice(t4 * 1024, (t4 + 1) * 1024)
            nc.sync.dma_start(out=xs[k][:, cs], in_=xT[k * 128 : (k + 1) * 128, cs])
    ident = const.tile([128, 128], F16)
    make_identity(nc, ident)

    q_rot = const.tile([128, 4096], F16)
    k_rot = const.tile([128, 4096], F16)
    # V^T slab: per head (80-col groups), 32 token tiles of [128 tok, 64]
    # with a ones column at 64 (the 65th stationary column accumulates the
    # softmax denominator into PSUM row 64 during AV).
    v_slab = const.tile([128, 32, 160], F16, name="vslab")
    nc.vector.memset(v_slab, 1.0)

    at = tc.alloc_tile_pool(name="attn", bufs=1)
    pp = tc.alloc_tile_pool(name="phasep", bufs=1)

    # ---------------- phase P as backfill pieces ----------------
    def pp_pieces(t4, early):
        dma = nc.scalar if early else nc.sync  # rope swaps off the busy queue
        state = {}

        def proj_half(wname, h2):
            def f():
                if h2 == 0:
                    state["ps"] = psum.tile(
                        [128, 1024], F32, tag="mm", bufs=3, name="ps_pr"
                    )
                ps = state["ps"]
                for k in range(8):
                    nc.tensor.matmul(
                        ps[:, h2 * 512 : (h2 + 1) * 512],
                        lhsT=w_sb[wname][:, k, :],
                        rhs=xs[k][
                            :, t4 * 1024 + h2 * 512 : t4 * 1024 + (h2 + 1) * 512
                        ],
                        start=(k == 0),
                        stop=(k == 7),
                    )

            return f

        def rope(dst):
            def f():
                # dst[:, cs] = ps*A + swap(ps*Bpre)
                ps = state.pop("ps")
                cs = slice(t4 * 1024, (t4 + 1) * 1024)
                rs = slice((t4 % 2) * 1024, (t4 % 2) * 1024 + 1024)
                t1 = pp.tile([128, 1024], F16, tag="t1", bufs=1, name="ropet1")
                nc.vector.tensor_tensor(t1, ps, rA[:, rs], op=Mul)
                r2 = pp.tile([128, 1024], F16, tag="r2", bufs=1, name="roper2")
                nc.vector.tensor_tensor(r2, ps, rB[:, rs], op=Mul)
                sw = pp.tile([128, 1024], F16, tag="sw", bufs=1, name="ropesw")
                for dst_p, src_p in ((0, 32), (32, 0), (64, 96), (96, 64)):
                    dma.dma_start(
                        out=sw[dst_p : dst_p + 32, :], in_=r2[src_p : src_p + 32, :]
                    )
                nc.vector.tensor_tensor(dst[:, cs], t1, sw, op=Add)

            return f

        def v_fin():
            # evict V, then transpose on the PE per 128-token tile; both
            # heads land in one strided DVE copy (ones columns kept)
            ps = state.pop("ps")
            vt_raw = pp.tile([128, 1024], F16, tag="vt", bufs=2, name="vtraw")
            nc.vector.tensor_copy(vt_raw, ps)
            for i in range(8):
                pst = psum.tile([128, 1024], F32, tag="mm", bufs=3, name="ps_tr")
                tr = pst[:, 0:64].bitcast(F16)  # [128, 128] f16 view
                nc.tensor.transpose(tr, vt_raw[:, i * 128 : (i + 1) * 128], ident)
                nc.vector.tensor_copy(
                    v_slab[:, t4 * 8 + i, :].rearrange("p (h e) -> p h e", h=2)[
                        :, :, 0:64
                    ],
                    tr.rearrange("p (h e) -> p h e", h=2),
                )

        return [
            proj_half("wq", 0),
            proj_half("wq", 1),
            rope(q_rot),
            proj_half("wk", 0),
            proj_half("wk", 1),
            rope(k_rot),
            proj_half("wv", 0),
            proj_half("wv", 1),
            v_fin,
        ]

    # ---------------- attention ----------------
    backfill = []  # deferred PE pieces (projections, output projection)
    prev_fin = [None]  # second-AV-pass + eviction closure of previous chunk

    def chunk(b, qh, pop2_until=0):
        qoff = b * T + qh * QCH

        exp_tiles = {}

        def s_exp(kt):
            koff = b * T + kt * 128
            for hi in range(2):
                pss = psum.tile([128, 1024], F32, tag="mm", bufs=3, name="ps_s")
                hs = slice(64 * hi, 64 * hi + 64)
                for h2 in range(2):
                    nc.tensor.matmul(
                        pss[:, h2 * 512 : (h2 + 1) * 512],
                        lhsT=k_rot[hs, koff : koff + 128],
                        rhs=q_rot[hs, qoff + h2 * 512 : qoff + (h2 + 1) * 512],
                        start=True,
                        stop=True,
                    )
                e = at.tile([128, 1024], F16, tag="exp", bufs=32, name="exps")
                nc.scalar.activation(e, pss, Exp, scale=0.125)
                exp_tiles[(hi, kt)] = e

        def av(ps_o, h2, kt):
            # ps_o: per head, a [65, 512] fp32 bank (rows 0:64 = O^T,
            # row 64 = the ones-column softmax denominator)
            h2s = slice(h2 * 512, (h2 + 1) * 512)
            for hi in range(2):
                e = exp_tiles[(hi, kt)]
                if h2 == 1:
                    del exp_tiles[(hi, kt)]
                nc.tensor.matmul(
                    ps_o[hi][:, :],
                    lhsT=v_slab[:, b * KT_N + kt, 80 * hi : 80 * hi + 65],
                    rhs=e[:, h2s],
                    start=(kt == 0),
                    stop=(kt == KT_N - 1),
                    skip_group_check=True,
                )

        # ocat [128, 1024] = both heads' normalized O^T for this chunk
        ocat = at.tile([128, 1024], F16, tag="ocat", bufs=2, name="ocat")
        oBt = at.tile([64, 1024], F16, tag="oBt", bufs=2, name="oBt")

        def evict(ps_o, h2):
            # rec = 1/den (approx), partition-broadcast, multiply folded
            # into the O^T eviction; head 1 via oBt + partition-move DMA.
            h2s = slice(h2 * 512, (h2 + 1) * 512)
            for hi in range(2):
                dent = at.tile([1, 512], F32, tag="dent", bufs=2, name="dent")
                nc.vector.tensor_copy(dent, ps_o[hi][64:65, :])
                rec = at.tile([1, 512], F32, tag="rec", bufs=2, name="rec")
                nc.vector.reciprocal_approx_fast(out=rec, in_=dent)
                rec_b = at.tile([64, 512], F32, tag="recb", bufs=2, name="recb")
                nc.gpsimd.partition_broadcast(rec_b, rec)
                dst = ocat[0:64, h2s] if hi == 0 else oBt[:, h2s]
                nc.vector.tensor_tensor(dst, ps_o[hi][0:64, :], rec_b, op=Mul)

        # ---- emission ----
        s_exp(0)
        s_exp(1)
        if prev_fin[0] is not None:
            prev_fin[0]()  # previous chunk: AV pass 2 + eviction + oproj queue
            prev_fin[0] = None
        ps_o1 = [
            psum.tile([65, 512], F32, tag="o", bufs=2, name=f"ps_o1{hi}")
            for hi in range(2)
        ]
        av(ps_o1, 0, 0)
        for kt in range(2, KT_N):
            s_exp(kt)
            if backfill:
                backfill.pop(0)()
            if kt <= pop2_until and backfill:
                backfill.pop(0)()
            av(ps_o1, 0, kt - 1)
        av(ps_o1, 0, KT_N - 1)
        evict(ps_o1, 0)

        def fin(b=b, qoff=qoff, ocat=ocat, oBt=oBt):
            # pass 2 rotates into pass 1's freed banks
            ps_o2 = [
                psum.tile([65, 512], F32, tag="o", bufs=2, name=f"ps_o2{hi}")
                for hi in range(2)
            ]
            for kt in range(KT_N):
                av(ps_o2, 1, kt)
            evict(ps_o2, 1)
            nc.sync.dma_start(out=ocat[64:128, :], in_=oBt)
            for nt in range(8):

                def piece(nt=nt, ocat=ocat, qoff=qoff, on_act=False):
                    nts = slice(nt * 128, (nt + 1) * 128)
                    ps_u = psum.tile([128, 1024], F32, tag="mm", bufs=3, name="ps_u")
                    for h2 in range(2):
                        h2s = slice(h2 * 512, (h2 + 1) * 512)
                        nc.tensor.matmul(
                            ps_u[:, h2s],
                            lhsT=wo_sb[:, nts],
                            rhs=ocat[:, h2s],
                            start=True,
                            stop=True,
                        )
                    if on_act:
                        ot = at.tile([128, 1024], F16, tag="ota", bufs=1, name="ota")
                        nc.scalar.copy(ot, ps_u)
                    else:
                        ot = at.tile([128, 1024], F16, tag="otv", bufs=2, name="otv")
                        nc.vector.tensor_copy(ot, ps_u)
                    nc.sync.dma_start(out=outT[nts, qoff : qoff + QCH], in_=ot)

                backfill.append(piece)

        prev_fin[0] = fin

    # ---------------- schedule ----------------
    # pp0 runs inline; chunk(0,0) kt 0-7 only needs batch-0/t4=0 keys, so
    # pp1 (and then pp2) backfill inside its kt loop, two pieces per kt.
    for f in pp_pieces(0, early=True):
        f()
    for f in pp_pieces(1, early=True):
        f()
    backfill.extend(pp_pieces(2, early=False))
    chunk(0, 0)
    backfill.extend(pp_pieces(3, early=False))
    chunk(0, 1)
    chunk(1, 0)
    chunk(1, 1)
    # tail: run remaining backfill, then the last chunk's finalization;
    # exp is done so half the oproj evictions ride the Act queue
    while backfill:
        backfill.pop(0)()
    prev_fin[0]()
    prev_fin[0] = None
    for j, piece in enumerate(backfill):
        piece(on_act=(j % 2 == 1))
    backfill.clear()

    pp.release()
    at.release()
    const.release()
    psum.release()


_NC_CACHE = {}


def _build_program():
    if 0 in _NC_CACHE:
        return _NC_CACHE[0]
    nc = bacc.Bacc("TRN2", num_devices=N_CORES, debug=False)
    xT = nc.dram_tensor("xT", [D, NTOK], F16, kind="ExternalInput").ap()
    wqT = nc.dram_tensor("wqT", [D, 128], F16, kind="ExternalInput").ap()
    wkT = nc.dram_tensor("wkT", [D, 128], F16, kind="ExternalInput").ap()
    wvT = nc.dram_tensor("wvT", [D, 128], F16, kind="ExternalInput").ap()
    woT = nc.dram_tensor("woT", [128, D], F16, kind="ExternalInput").ap()
    ropeA = nc.dram_tensor("ropeA", [128, T], F16, kind="ExternalInput").ap()
    ropeB = nc.dram_tensor("ropeB", [128, T], F16, kind="ExternalInput").ap()
    outT = nc.dram_tensor("outT", [D, NTOK], F16, kind="ExternalOutput").ap()
    with tile.TileContext(nc) as tc:
        _build_body(tc, xT, wqT, wkT, wvT, woT, ropeA, ropeB, outT)
    nc.compile()
    _NC_CACHE[0] = nc
    return nc


def _rope_tables():
    half = DK // 2  # 32
    inv_freq = 1.0 / (
        10000.0 ** (np.arange(0, DK, 2, dtype=np.float32) / np.float32(DK))
    )
    t = np.arange(T, dtype=np.float32)
    freqs = np.outer(t, inv_freq)  # [T, 32]
    cos = np.cos(freqs)
    sin = np.sin(freqs)
    A = np.empty((128, T), np.float32)
    Bp = np.empty((128, T), np.float32)
    for p in range(128):
        i = p % DK
        if i < half:
            A[p] = cos[:, i]
            Bp[p] = sin[:, i]  # pre-swapped: lands at row i+32 after the swap
        else:
            A[p] = cos[:, i - half]
            Bp[p] = -sin[:, i - half]  # lands at row i-32 after the swap
    return A.astype(np.float16), Bp.astype(np.float16)


def _prep_inputs(x, wq, wk, wv, wo):
    xT = np.ascontiguousarray(x.reshape(NTOK, D).T).astype(np.float16)
    ropeA, ropeB = _rope_tables()
    in_maps = []
    for c in range(N_CORES):
        rows = slice(128 * c, 128 * (c + 1))
        in_maps.append(
            {
                "xT": xT,
                "wqT": np.ascontiguousarray(wq[rows, :].T).astype(np.float16),
                "wkT": np.ascontiguousarray(wk[rows, :].T).astype(np.float16),
                "wvT": np.ascontiguousarray(wv[rows, :].T).astype(np.float16),
                "woT": np.ascontiguousarray(wo[:, rows].T).astype(np.float16),
                "ropeA": ropeA,
                "ropeB": ropeB,
            }
        )
    return in_maps


def run(x, wq, wk, wv, wo, trace=False):
    """Returns (output (B,T,D) fp32, BassKernelResults)."""
    from concourse import bass_utils

    nc = _build_program()
    in_maps = _prep_inputs(
        np.asarray(x, np.float32),
        np.asarray(wq, np.float32),
        np.asarray(wk, np.float32),
        np.asarray(wv, np.float32),
        np.asarray(wo, np.float32),
    )
    res = bass_utils.run_bass_kernel_spmd(
        nc, in_maps, core_ids=list(range(N_CORES)), trace=trace
    )
    acc = np.zeros((D, NTOK), np.float32)
    for c in range(N_CORES):
        acc += np.asarray(res.results[c]["outT"], np.float32)
    out = acc.T.reshape(B, T, D)
    return out, res


def kernel(x, wq, wk, wv, wo):
    out, _ = run(x, wq, wk, wv, wo)
    return out
